# revision 8
# baseline (speedup 1.0000x reference)
"""Trainium2 Bass kernel for nn_EntityLinker — two-stage gather architecture.

Stage 1: chunk-sorted dma_gather (int16 idx, 4 table chunks) -> SBUF staging.
Stage 2: dma_scatter_add (SBUF parity dest, CCE add) un-permutes staged rows
into (p, pair) cell layout and performs the T-token sum for free.
Round structure keeps cells unique per scatter call (CCE RMW races otherwise).
Compute: per-pair attention + MLP identical to the v1 kernel.
"""

import sys

if "/opt/trn_rl_repo" not in sys.path:
    sys.path.insert(0, "/opt/trn_rl_repo")

import numpy as np

V, D = 100000, 128
B, Q, C, T = 1024, 64, 64, 8
NCORES = 8
BL = B // NCORES          # 128 batches per core
PAIRS = BL // 2           # 64 pairs
NQUAD = 4                 # quarters of 16 pairs
PQ = PAIRS // NQUAD       # 16 pairs per quarter
CHUNK = 32768
NCHUNK = 4
GSIZE = 1024              # idx per gather call
SSIZE = 1024              # idx per scatter call
NEG = np.float32(-1.0e30)
SCALE_SIM = float(1.0 / np.sqrt(128.0))

_P_H = np.arange(128) // 64
_P_C = np.arange(128) % 64

_BLOCKIND = np.zeros((2, 128), np.float32)
_BLOCKIND[0, :64] = 1.0
_BLOCKIND[1, 64:] = 1.0


def _wrap16(lst):
    """idx list -> [128, n/16] int16, replicated 8x over partition groups."""
    a = np.asarray(lst, np.int16).reshape(-1, 16).T
    return np.tile(a, (8, 1)).copy()


def _prep_core(core, q_ids, c_ids, num_qs, cnt):
    """Build gather/scatter call plans + masks for one core.

    Returns dict with:
      gidx   [128, GT/16] i16   - gather idx lists (concatenated calls)
      sidx   [128, ST/16] i16   - scatter idx lists (concatenated calls)
      cscale [128, PAIRS] f32, qv [128, 2*PAIRS] f32, qbias [2, PAIRS*128] f32
      plan: python list of call descriptors (shared across cores by padding)
    """
    base = core * BL
    # --- token lists ---------------------------------------------------
    # c-token: cell = pair*128 + (h*64 + c), 8 tokens each
    # q-token: cell = pair*128 + (h*64 + qpos), 1 token each
    pair = np.arange(PAIRS)
    toks = []  # (chunk, kind(0=c,1=q), cell, id)
    for pj in pair:
        for h in (0, 1):
            b = base + 2 * pj + h
            pbase = h * 64
            ids_c = c_ids[b]            # [64, 8]
            for cc in range(C):
                cell = pj * 128 + pbase + cc
                for t in range(T):
                    i = int(ids_c[cc, t])
                    toks.append((i // CHUNK, 0, cell, i))
            ids_q = q_ids[b]            # [64]
            for qq in range(Q):
                cell = pj * 128 + pbase + qq
                i = int(ids_q[qq])
                toks.append((i // CHUNK, 1, cell, i))
    # round assignment per (chunk, cell) for c-tokens (q tokens unique/cell)
    from collections import defaultdict
    rc = defaultdict(int)
    rec = []
    for (ch, kind, cell, i) in toks:
        if kind == 0:
            r = rc[(ch, cell)]
            rc[(ch, cell)] += 1
        else:
            r = 0
        rec.append((ch, kind, r, cell >> 12, cell, i))
    rec.sort(key=lambda x: (x[3], x[0], x[1], x[2]))
    return rec


def _plan_and_pack(recs_all):
    """Slab = (quarter, chunk, kind, round). Pad each slab to 128 and slice
    into calls. All cores share ONE program -> slab sizes = max over cores,
    shorter cores pad with dummy tokens (gather idx 0, scatter idx -1)."""
    from collections import defaultdict
    keys = set()
    percore = []
    for rec in recs_all:
        g = defaultdict(list)
        for (ch, kind, r, half, cell, i) in rec:
            g[(half, ch, kind, r)].append((cell, i))
        percore.append(g)
        keys |= set(g.keys())
    keys = sorted(keys)
    slab_n = {k: max(len(g.get(k, ())) for g in percore) for k in keys}
    # pad slabs to multiple of 128
    slab_n = {k: -(-n // 128) * 128 for k, n in slab_n.items()}

    total = 0
    callmeta = []   # per GATHER piece: dict(chunk, n, pos, subs=[...])
    setctr = defaultdict(int)
    cur = None
    for k in keys:
        half, ch, kind, r = k
        n = slab_n[k]
        off = 0
        while off < n:
            if cur is not None and (cur["chunk"] != ch or cur["n"] >= GSIZE):
                callmeta.append(cur)
                cur = None
            if cur is None:
                cur = dict(chunk=ch, n=0, pos=total, subs=[])
            take = min(n - off, GSIZE - cur["n"], SSIZE)
            s = 0
            cur["subs"].append(dict(kind=kind, set=s, half=half, n=take,
                                    pos=total))
            cur["n"] += take
            total += take
            off += take
    if cur is not None:
        callmeta.append(cur)
    # build per-core idx arrays
    gidx_all, sidx_all = [], []
    for g in percore:
        gl = np.zeros(total, np.int16)
        sl = np.zeros(total, np.int16)
        cursor = 0
        for k in keys:
            half, ch, kind, r = k
            items = g.get(k, [])
            n = slab_n[k]
            for j in range(n):
                if j < len(items):
                    cell, i = items[j]
                    gl[cursor + j] = np.int16(i - ch * CHUNK)
                    # cell relative to its half slice (16 groups + trash@16)
                    lp = cell - half * 4096
                    sl[cursor + j] = np.int16(lp)
                else:
                    gl[cursor + j] = 0
                    sl[cursor + j] = np.int16(32 * 128 + (j % 128))
            cursor += n
        gidx_all.append(gl)
        sidx_all.append(sl)
    return callmeta, total, gidx_all, sidx_all


def prep_all(q_ids, c_ids, num_qs):
    q_ids = np.asarray(q_ids).astype(np.int64)
    c_ids = np.asarray(c_ids).astype(np.int64)
    num_qs = np.asarray(num_qs).astype(np.int64)
    cnt = np.maximum((c_ids != 0).sum(-1), 1).astype(np.float32)

    recs = [_prep_core(i, q_ids, c_ids, num_qs, cnt) for i in range(NCORES)]
    callmeta, total, gidx_all, sidx_all = _plan_and_pack(recs)

    # masks per core (same layout as v1 kernel but full-core tensors)
    outs = []
    for core in range(NCORES):
        base = core * BL
        cscale = np.empty((128, PAIRS), np.float32)
        qv = np.zeros((128, 2 * PAIRS), np.float32)
        qbias = np.empty((2, PAIRS * 128), np.float32)
        for pj in range(PAIRS):
            bmat = base + 2 * pj + _P_H          # [128]
            cscale[:, pj] = 1.0 / cnt[bmat, _P_C]
            for r in range(2):
                b = base + 2 * pj + r
                nq = int(num_qs[b])
                blk = np.full(128, NEG, np.float32)
                blk[r * 64:r * 64 + nq] = 0.0
                qbias[r, pj * 128:(pj + 1) * 128] = blk
                valid = (_P_C < nq) & (_P_H == r)
                qv[:, 2 * pj + r] = valid / np.float32(nq)
        outs.append(dict(
            gidx=_wrap16(gidx_all[core]),
            sidx=_wrap16(sidx_all[core]),
            cscale=cscale, qv=qv, qbias=qbias))
    return outs, callmeta, total


def _build_program(callmeta, total):
    from contextlib import ExitStack

    import concourse.bass as bass
    from concourse import bacc, mybir, tile
    from concourse.masks import make_identity

    f32 = mybir.dt.float32
    i16 = mybir.dt.int16

    nc = bacc.Bacc("TRN2", target_bir_lowering=False, debug=False,
                   enable_asserts=False, num_devices=NCORES)

    embed_d = nc.dram_tensor("embed", [V, D], f32, kind="ExternalInput").ap()
    w_h_d = nc.dram_tensor("w_h", [5 * D, D], f32, kind="ExternalInput").ap()
    w_o_d = nc.dram_tensor("w_o", [D, 1], f32, kind="ExternalInput").ap()
    b_h_d = nc.dram_tensor("b_h", [D, 1], f32, kind="ExternalInput").ap()
    b_o_d = nc.dram_tensor("b_o_bc", [D, 1], f32, kind="ExternalInput").ap()
    blockind_d = nc.dram_tensor("blockind", [2, 128], f32, kind="ExternalInput").ap()
    gidx_d = nc.dram_tensor("gidx", [128, total // 16], i16, kind="ExternalInput").ap()
    sidx_d = nc.dram_tensor("sidx", [128, total // 16], i16, kind="ExternalInput").ap()
    cscale_d = nc.dram_tensor("cscale", [128, PAIRS], f32, kind="ExternalInput").ap()
    qv_d = nc.dram_tensor("qv", [128, 2 * PAIRS], f32, kind="ExternalInput").ap()
    qbias_d = nc.dram_tensor("qbias", [2, PAIRS * 128], f32, kind="ExternalInput").ap()
    out_d = nc.dram_tensor("out", [PAIRS, BL], f32, kind="ExternalOutput").ap()

    with tile.TileContext(nc) as tc, ExitStack() as ctx:
        const = ctx.enter_context(tc.tile_pool(name="const", bufs=1))
        acc = ctx.enter_context(tc.tile_pool(name="acc", bufs=1))
        stp = ctx.enter_context(tc.tile_pool(name="stage", bufs=5))
        spool = ctx.enter_context(tc.tile_pool(name="work", bufs=4))
        ppool = ctx.enter_context(tc.tile_pool(name="psum", bufs=1, space="PSUM"))

        ident = const.tile([128, 128], f32)
        make_identity(nc, ident[:])
        whk = const.tile([128, 5 * 128], f32)
        for k in range(5):
            nc.sync.dma_start(whk[:, k * 128:(k + 1) * 128],
                              w_h_d[k * 128:(k + 1) * 128, :])
        w_o_t = const.tile([128, 1], f32)
        nc.sync.dma_start(w_o_t[:], w_o_d[:])
        b_h_t = const.tile([128, 1], f32)
        nc.sync.dma_start(b_h_t[:], b_h_d[:])
        b_o_t = const.tile([128, 1], f32)
        nc.sync.dma_start(b_o_t[:], b_o_d[:])
        blockind_t = const.tile([2, 128], f32)
        nc.sync.dma_start(blockind_t[:], blockind_d[:])
        gidx_t = const.tile([128, total // 16], i16)
        nc.sync.dma_start(gidx_t[:], gidx_d[:])
        sidx_t = const.tile([128, total // 16], i16)
        nc.sync.dma_start(sidx_t[:], sidx_d[:])
        cscale_t = const.tile([128, PAIRS], f32)
        nc.sync.dma_start(cscale_t[:], cscale_d[:])
        qv_t = const.tile([128, 2 * PAIRS], f32)
        nc.sync.dma_start(qv_t[:], qv_d[:])
        qbias_t = const.tile([2, PAIRS * 128], f32)
        nc.sync.dma_start(qbias_t[:], qbias_d[:])
        out_sb = const.tile([128, PAIRS], f32)

        # accumulators: c (2 sets) and q (1 set), each own(even pair)/peer(odd)
        # layout: [half0 groups0-15, half0 trash, half1 groups, half1 trash]
        NG34 = (PAIRS // 2 + 2) * 128
        HSL = 17 * 128       # one half slice incl. its trash group
        cacc = [[acc.tile([128, NG34], f32, name=f"cacc{s}{p}")
                 for p in range(2)] for s in range(1)]      # [set][parity]
        qacc = [acc.tile([128, NG34], f32, name=f"qacc{p}") for p in range(2)]
        nc.vector.memset(cacc[0][0][:], 0.0)
        nc.vector.memset(qacc[0][:], 0.0)
        nc.scalar.memzero(cacc[0][1][:])
        nc.scalar.memzero(qacc[1][:])

        Act = mybir.ActivationFunctionType

        # ---- gather/scatter pipeline --------------------------------------
        ncalls = len(callmeta)
        stages = [None] * ncalls

        def emit_gather(ci):
            m = callmeta[ci]
            n = m["n"]
            st = stp.tile([128, GSIZE // 128 * 128], f32, tag="st")
            ch = m["chunk"]
            hi = min((ch + 1) * CHUNK, V)
            nc.gpsimd.dma_gather(
                out_ap=st[:, :n].rearrange("p (n d) -> p n d", d=D),
                in_ap=embed_d[ch * CHUNK:hi, :],
                idxs_ap=gidx_t[:, m["pos"] // 16:(m["pos"] + n) // 16],
                num_idxs=n, num_idxs_reg=n, elem_size=D)
            stages[ci] = st

        def emit_scatter(ci):
            m = callmeta[ci]
            st = stages[ci]
            for sub in m["subs"]:
                n = sub["n"]
                rel = sub["pos"] - m["pos"]
                hs = sub["half"] * HSL
                if sub["kind"] == 0:
                    own = cacc[sub["set"]][0][:, hs:hs + HSL]
                    peer = cacc[sub["set"]][1][:, hs:hs + HSL]
                else:
                    own = qacc[0][:, hs:hs + HSL]
                    peer = qacc[1][:, hs:hs + HSL]
                nc.gpsimd.dma_scatter_add(
                    out_ap=own.rearrange("p (n d) -> p n d", d=D),
                    in_ap=st[:, rel:rel + n]
                        .rearrange("p (n d) -> p n d", d=D),
                    idxs_ap=sidx_t[:, sub["pos"] // 16:(sub["pos"] + n) // 16],
                    num_idxs=n, num_idxs_reg=n, elem_size=D,
                    sbuf_tokens_per_rank=128, parity_reg=0,
                    out_ap_other=peer.rearrange("p (n d) -> p n d", d=D))
            stages[ci] = None

        LOOKAHEAD = 2
        for ci in range(ncalls):
            emit_gather(ci)
            if ci >= LOOKAHEAD:
                emit_scatter(ci - LOOKAHEAD)
        for ci in range(max(0, ncalls - LOOKAHEAD), ncalls):
            emit_scatter(ci)

        # ---- compute (identical math to v1, c_sum = A+B sets) -------------
        for pj in range(PAIRS):
            par = pj % 2
            hf = pj // 32
            grp = (pj - 32 * hf) // 2 + 17 * hf
            csl = slice(grp * 128, (grp + 1) * 128)
            q_h2 = qacc[par][:, csl]

            c_h2 = spool.tile([128, 128], f32, tag="c_h2")
            nc.vector.tensor_scalar_mul(c_h2[:], cacc[0][par][:, csl],
                                        cscale_t[:, pj:pj + 1])

            t1 = ppool.tile([128, 128], f32, tag="ps_t1")
            nc.tensor.transpose(t1[:], c_h2[:], ident[:])
            c_hT = spool.tile([128, 128], f32, tag="c_hT")
            nc.vector.tensor_copy(c_hT[:], t1[:])

            t2 = ppool.tile([128, 128], f32, tag="ps_t2")
            nc.tensor.transpose(t2[:], q_h2, ident[:])
            q_hT = spool.tile([128, 128], f32, tag="q_hT")
            nc.scalar.copy(q_hT[:], t2[:])

            sim = ppool.tile([128, 128], f32, tag="ps_sim")
            nc.tensor.matmul(sim[:], lhsT=c_hT[:], rhs=q_hT[:],
                             start=True, stop=False)
            nc.tensor.matmul(sim[:], lhsT=blockind_t[:],
                             rhs=qbias_t[:, pj * 128:(pj + 1) * 128],
                             start=False, stop=True)

            att_e = spool.tile([128, 128], f32, tag="att_e")
            s_col = spool.tile([128, 1], f32, tag="s_col")
            nc.scalar.activation(att_e[:], sim[:], Act.Exp,
                                 scale=SCALE_SIM, accum_out=s_col[:])
            r_col = spool.tile([128, 1], f32, tag="r_col")
            nc.vector.reciprocal(r_col[:], s_col[:])
            att = spool.tile([128, 128], f32, tag="att")
            nc.vector.tensor_scalar_mul(att[:], att_e[:], r_col[:])

            t3 = ppool.tile([128, 128], f32, tag="ps_t3")
            nc.tensor.transpose(t3[:], att[:], ident[:])
            attT = spool.tile([128, 128], f32, tag="attT")
            nc.scalar.copy(attT[:], t3[:])

            wq_ps = ppool.tile([128, 128], f32, tag="ps_wq")
            nc.tensor.matmul(wq_ps[:], lhsT=q_h2, rhs=attT[:],
                             start=True, stop=True)
            wqT = spool.tile([128, 128], f32, tag="wqT")
            nc.vector.tensor_copy(wqT[:], wq_ps[:])

            sm_ps = ppool.tile([128, 16], f32, tag="ps_small")
            qs_ps = sm_ps[:, 0:2]
            nc.tensor.matmul(qs_ps, lhsT=q_h2,
                             rhs=qv_t[:, pj * 2:(pj + 1) * 2],
                             start=True, stop=True)
            qs_sb = spool.tile([128, 2], f32, tag="qs_sb")
            nc.vector.tensor_copy(qs_sb[:], qs_ps)

            bias_ps = sm_ps[:, 4:6]
            nc.tensor.matmul(bias_ps, lhsT=whk[:, 0:128], rhs=qs_sb[:],
                             start=True, stop=True)
            bias_sb = spool.tile([128, 2], f32, tag="bias_sb")
            nc.scalar.activation(bias_sb[:], bias_ps, Act.Identity,
                                 bias=b_h_t[:, 0:1])

            ch3 = spool.tile([128, 128], f32, tag="ch3")
            nc.vector.tensor_mul(ch3[:], c_hT[:], wqT[:])
            dif = spool.tile([128, 128], f32, tag="dif")
            nc.vector.tensor_sub(dif[:], c_hT[:], wqT[:])
            ch4 = spool.tile([128, 128], f32, tag="ch4")
            nc.scalar.activation(ch4[:], dif[:], Act.Abs)

            h_ps = ppool.tile([128, 128], f32, tag="ps_h")
            for k, rhs in ((1, c_hT), (2, wqT), (3, ch3), (4, ch4)):
                nc.tensor.matmul(h_ps[:], lhsT=whk[:, k * 128:(k + 1) * 128],
                                 rhs=rhs[:], start=(k == 1), stop=(k == 4))
            hT = spool.tile([128, 128], f32, tag="hT")
            for r in range(2):
                nc.scalar.activation(hT[:, r * 64:(r + 1) * 64],
                                     h_ps[:, r * 64:(r + 1) * 64], Act.Tanh,
                                     bias=bias_sb[:, r:r + 1])

            o_ps = sm_ps[:, 8:9]
            nc.tensor.matmul(o_ps, lhsT=hT[:], rhs=w_o_t[:],
                             start=True, stop=True)
            nc.scalar.activation(out_sb[:, pj:pj + 1], o_ps, Act.Identity,
                                 bias=b_o_t[:, 0:1])

        ot_ps = ppool.tile([PAIRS, 128], f32, tag="ps_ot")
        nc.tensor.transpose(ot_ps[:], out_sb[:], ident[:])
        out_f = const.tile([PAIRS, 128], f32)
        nc.vector.tensor_copy(out_f[:], ot_ps[:])
        nc.sync.dma_start(out_d[:], out_f[:])

    nc.compile()
    return nc


def make_in_maps(q_ids, c_ids, num_qs, num_cols, embed, W_h, b_h, W_o, b_o):
    embed = np.ascontiguousarray(np.asarray(embed, np.float32))
    W_h = np.ascontiguousarray(np.asarray(W_h, np.float32))
    W_o = np.ascontiguousarray(np.asarray(W_o, np.float32).reshape(D, 1))
    b_h = np.ascontiguousarray(np.asarray(b_h, np.float32).reshape(D, 1))
    b_o_bc = np.full((D, 1), np.float32(np.asarray(b_o).reshape(-1)[0]))
    shared = dict(embed=embed, w_h=W_h, w_o=W_o, b_h=b_h, b_o_bc=b_o_bc,
                  blockind=_BLOCKIND)
    percore, callmeta, total = prep_all(q_ids, c_ids, num_qs)
    in_maps = [dict(shared, **percore[i]) for i in range(NCORES)]
    return in_maps, callmeta, total


_PROGRAM = None
_PROGKEY = None


def kernel(q_ids, c_ids, num_qs, num_cols, embed, W_h, b_h, W_o, b_o):
    global _PROGRAM, _PROGKEY
    in_maps, callmeta, total = make_in_maps(q_ids, c_ids, num_qs, num_cols,
                                            embed, W_h, b_h, W_o, b_o)
    key = (len(callmeta), total)
    if _PROGRAM is None or _PROGKEY != key:
        _PROGRAM = _build_program(callmeta, total)
        _PROGKEY = key
    from concourse import bass_utils
    res = bass_utils.run_bass_kernel_spmd(
        _PROGRAM, in_maps, core_ids=list(range(NCORES)), trace=False)
    outs = np.empty((B, C, 1), np.float32)
    for i in range(NCORES):
        outs[i * BL:(i + 1) * BL, :, 0] = res.results[i]["out"].reshape(BL, C)
    return outs


# revision 9
# speedup vs baseline: 1.2717x; 1.2717x over previous
"""Trainium2 Bass kernel for nn_EntityLinker — two-stage gather architecture.

Stage 1: chunk-sorted dma_gather (int16 idx, 4 table chunks) -> SBUF staging.
Stage 2: dma_scatter_add (SBUF parity dest, CCE add) un-permutes staged rows
into (p, pair) cell layout and performs the T-token sum for free.
Round structure keeps cells unique per scatter call (CCE RMW races otherwise).
Compute: per-pair attention + MLP identical to the v1 kernel.
"""

import sys

if "/opt/trn_rl_repo" not in sys.path:
    sys.path.insert(0, "/opt/trn_rl_repo")

import numpy as np

V, D = 100000, 128
B, Q, C, T = 1024, 64, 64, 8
NCORES = 8
BL = B // NCORES          # 128 batches per core
PAIRS = BL // 2           # 64 pairs
NQUAD = 4                 # quarters of 16 pairs
PQ = PAIRS // NQUAD       # 16 pairs per quarter
CHUNK = 32768
NCHUNK = 4
GSIZE = 1024              # idx per gather call
SSIZE = 1024              # idx per scatter call
NEG = np.float32(-1.0e30)
SCALE_SIM = float(1.0 / np.sqrt(128.0))

_P_H = np.arange(128) // 64
_P_C = np.arange(128) % 64

_BLOCKIND = np.zeros((2, 128), np.float32)
_BLOCKIND[0, :64] = 1.0
_BLOCKIND[1, 64:] = 1.0


def _wrap16(lst):
    """idx list -> [128, n/16] int16, replicated 8x over partition groups."""
    a = np.asarray(lst, np.int16).reshape(-1, 16).T
    return np.tile(a, (8, 1)).copy()


def _prep_core(core, q_ids, c_ids, num_qs, cnt):
    """Build gather/scatter call plans + masks for one core.

    Returns dict with:
      gidx   [128, GT/16] i16   - gather idx lists (concatenated calls)
      sidx   [128, ST/16] i16   - scatter idx lists (concatenated calls)
      cscale [128, PAIRS] f32, qv [128, 2*PAIRS] f32, qbias [2, PAIRS*128] f32
      plan: python list of call descriptors (shared across cores by padding)
    """
    base = core * BL
    # --- token lists ---------------------------------------------------
    # c-token: cell = pair*128 + (h*64 + c), 8 tokens each
    # q-token: cell = pair*128 + (h*64 + qpos), 1 token each
    pair = np.arange(PAIRS)
    toks = []  # (chunk, kind(0=c,1=q), cell, id)
    for pj in pair:
        for h in (0, 1):
            b = base + 2 * pj + h
            pbase = h * 64
            ids_c = c_ids[b]            # [64, 8]
            for cc in range(C):
                cell = pj * 128 + pbase + cc
                for t in range(T):
                    i = int(ids_c[cc, t])
                    toks.append((i // CHUNK, 0, cell, i))
            ids_q = q_ids[b]            # [64]
            for qq in range(Q):
                cell = pj * 128 + pbase + qq
                i = int(ids_q[qq])
                toks.append((i // CHUNK, 1, cell, i))
    # round assignment per (chunk, cell) for c-tokens (q tokens unique/cell)
    from collections import defaultdict
    rc = defaultdict(int)
    rec = []
    for (ch, kind, cell, i) in toks:
        if kind == 0:
            r = rc[(ch, cell)]
            rc[(ch, cell)] += 1
        else:
            r = 0
        rec.append((ch, kind, r, cell >> 12, cell, i))
    rec.sort(key=lambda x: (x[3], x[0], x[1], x[2]))
    return rec


def _plan_and_pack(recs_all):
    """Slab = (quarter, chunk, kind, round). Pad each slab to 128 and slice
    into calls. All cores share ONE program -> slab sizes = max over cores,
    shorter cores pad with dummy tokens (gather idx 0, scatter idx -1)."""
    from collections import defaultdict
    keys = set()
    percore = []
    for rec in recs_all:
        g = defaultdict(list)
        for (ch, kind, r, half, cell, i) in rec:
            g[(half, ch, kind, r)].append((cell, i))
        percore.append(g)
        keys |= set(g.keys())
    keys = sorted(keys)
    slab_n = {k: max(len(g.get(k, ())) for g in percore) for k in keys}
    # pad slabs to multiple of 128
    slab_n = {k: -(-n // 128) * 128 for k, n in slab_n.items()}

    total = 0
    callmeta = []   # per GATHER piece: dict(chunk, n, pos, subs=[...])
    setctr = defaultdict(int)
    cur = None
    for k in keys:
        half, ch, kind, r = k
        n = slab_n[k]
        off = 0
        while off < n:
            if cur is not None and (cur["chunk"] != ch or cur["n"] >= GSIZE):
                callmeta.append(cur)
                cur = None
            if cur is None:
                cur = dict(chunk=ch, n=0, pos=total, subs=[])
            take = min(n - off, GSIZE - cur["n"], SSIZE)
            if kind == 0:
                s = setctr[(0, half)] % 2
                setctr[(0, half)] += 1
            else:
                s = 0
            cur["subs"].append(dict(kind=kind, set=s, half=half, n=take,
                                    pos=total))
            cur["n"] += take
            total += take
            off += take
    if cur is not None:
        callmeta.append(cur)
    # build per-core idx arrays
    gidx_all, sidx_all = [], []
    for g in percore:
        gl = np.zeros(total, np.int16)
        sl = np.zeros(total, np.int16)
        cursor = 0
        for k in keys:
            half, ch, kind, r = k
            items = g.get(k, [])
            n = slab_n[k]
            for j in range(n):
                if j < len(items):
                    cell, i = items[j]
                    gl[cursor + j] = np.int16(i - ch * CHUNK)
                    # cell relative to its half slice (16 groups + trash@16)
                    lp = cell - half * 4096
                    sl[cursor + j] = np.int16(lp)
                else:
                    gl[cursor + j] = 0
                    sl[cursor + j] = np.int16(32 * 128 + (j % 128))
            cursor += n
        gidx_all.append(gl)
        sidx_all.append(sl)
    return callmeta, total, gidx_all, sidx_all


def prep_all(q_ids, c_ids, num_qs):
    q_ids = np.asarray(q_ids).astype(np.int64)
    c_ids = np.asarray(c_ids).astype(np.int64)
    num_qs = np.asarray(num_qs).astype(np.int64)
    cnt = np.maximum((c_ids != 0).sum(-1), 1).astype(np.float32)

    recs = [_prep_core(i, q_ids, c_ids, num_qs, cnt) for i in range(NCORES)]
    callmeta, total, gidx_all, sidx_all = _plan_and_pack(recs)

    # masks per core (same layout as v1 kernel but full-core tensors)
    outs = []
    for core in range(NCORES):
        base = core * BL
        cscale = np.empty((128, PAIRS), np.float32)
        qv = np.zeros((128, 2 * PAIRS), np.float32)
        qbias = np.empty((2, PAIRS * 128), np.float32)
        for pj in range(PAIRS):
            bmat = base + 2 * pj + _P_H          # [128]
            cscale[:, pj] = 1.0 / cnt[bmat, _P_C]
            for r in range(2):
                b = base + 2 * pj + r
                nq = int(num_qs[b])
                blk = np.full(128, NEG, np.float32)
                blk[r * 64:r * 64 + nq] = 0.0
                qbias[r, pj * 128:(pj + 1) * 128] = blk
                valid = (_P_C < nq) & (_P_H == r)
                qv[:, 2 * pj + r] = valid / np.float32(nq)
        outs.append(dict(
            gidx=_wrap16(gidx_all[core]),
            sidx=_wrap16(sidx_all[core]),
            cscale=cscale, qv=qv, qbias=qbias))
    return outs, callmeta, total


def _build_program(callmeta, total):
    from contextlib import ExitStack

    import concourse.bass as bass
    from concourse import bacc, mybir, tile
    from concourse.masks import make_identity

    f32 = mybir.dt.float32
    i16 = mybir.dt.int16

    nc = bacc.Bacc("TRN2", target_bir_lowering=False, debug=False,
                   enable_asserts=False, num_devices=NCORES)

    embed_d = nc.dram_tensor("embed", [V, D], f32, kind="ExternalInput").ap()
    w_h_d = nc.dram_tensor("w_h", [5 * D, D], f32, kind="ExternalInput").ap()
    w_o_d = nc.dram_tensor("w_o", [D, 1], f32, kind="ExternalInput").ap()
    b_h_d = nc.dram_tensor("b_h", [D, 1], f32, kind="ExternalInput").ap()
    b_o_d = nc.dram_tensor("b_o_bc", [D, 1], f32, kind="ExternalInput").ap()
    blockind_d = nc.dram_tensor("blockind", [2, 128], f32, kind="ExternalInput").ap()
    gidx_d = nc.dram_tensor("gidx", [128, total // 16], i16, kind="ExternalInput").ap()
    sidx_d = nc.dram_tensor("sidx", [128, total // 16], i16, kind="ExternalInput").ap()
    cscale_d = nc.dram_tensor("cscale", [128, PAIRS], f32, kind="ExternalInput").ap()
    qv_d = nc.dram_tensor("qv", [128, 2 * PAIRS], f32, kind="ExternalInput").ap()
    qbias_d = nc.dram_tensor("qbias", [2, PAIRS * 128], f32, kind="ExternalInput").ap()
    out_d = nc.dram_tensor("out", [PAIRS, BL], f32, kind="ExternalOutput").ap()

    with tile.TileContext(nc) as tc, ExitStack() as ctx:
        const = ctx.enter_context(tc.tile_pool(name="const", bufs=1))
        acc = ctx.enter_context(tc.tile_pool(name="acc", bufs=1))
        stp = ctx.enter_context(tc.tile_pool(name="stage", bufs=5))
        spool = ctx.enter_context(tc.tile_pool(name="work", bufs=4))
        ppool = ctx.enter_context(tc.tile_pool(name="psum", bufs=1, space="PSUM"))

        ident = const.tile([128, 128], f32)
        make_identity(nc, ident[:])
        whk = const.tile([128, 5 * 128], f32)
        for k in range(5):
            nc.sync.dma_start(whk[:, k * 128:(k + 1) * 128],
                              w_h_d[k * 128:(k + 1) * 128, :])
        w_o_t = const.tile([128, 1], f32)
        nc.sync.dma_start(w_o_t[:], w_o_d[:])
        b_h_t = const.tile([128, 1], f32)
        nc.sync.dma_start(b_h_t[:], b_h_d[:])
        b_o_t = const.tile([128, 1], f32)
        nc.sync.dma_start(b_o_t[:], b_o_d[:])
        blockind_t = const.tile([2, 128], f32)
        nc.sync.dma_start(blockind_t[:], blockind_d[:])
        gidx_t = const.tile([128, total // 16], i16)
        nc.sync.dma_start(gidx_t[:], gidx_d[:])
        sidx_t = const.tile([128, total // 16], i16)
        nc.sync.dma_start(sidx_t[:], sidx_d[:])
        cscale_t = const.tile([128, PAIRS], f32)
        nc.sync.dma_start(cscale_t[:], cscale_d[:])
        qv_t = const.tile([128, 2 * PAIRS], f32)
        nc.sync.dma_start(qv_t[:], qv_d[:])
        qbias_t = const.tile([2, PAIRS * 128], f32)
        nc.sync.dma_start(qbias_t[:], qbias_d[:])
        out_sb = const.tile([128, PAIRS], f32)

        # accumulators: c (2 sets) and q (1 set), each own(even pair)/peer(odd)
        # layout: [half0 groups0-15, half0 trash, half1 groups, half1 trash]
        NG34 = (PAIRS // 2 + 2) * 128
        HSL = 17 * 128       # one half slice incl. its trash group
        cacc = [[acc.tile([128, NG34], f32, name=f"cacc{s}{p}")
                 for p in range(2)] for s in range(2)]      # [set][parity]
        qacc = [acc.tile([128, NG34], f32, name=f"qacc{p}") for p in range(2)]
        nc.vector.memset(cacc[0][0][:], 0.0)
        nc.vector.memset(cacc[1][0][:], 0.0)
        nc.vector.memset(qacc[0][:], 0.0)
        nc.scalar.memzero(cacc[0][1][:])
        nc.scalar.memzero(cacc[1][1][:])
        nc.scalar.memzero(qacc[1][:])

        Act = mybir.ActivationFunctionType

        # ---- gather/scatter pipeline --------------------------------------
        ncalls = len(callmeta)
        stages = [None] * ncalls

        def emit_gather(ci):
            m = callmeta[ci]
            n = m["n"]
            st = stp.tile([128, GSIZE // 128 * 128], f32, tag="st")
            ch = m["chunk"]
            hi = min((ch + 1) * CHUNK, V)
            nc.gpsimd.dma_gather(
                out_ap=st[:, :n].rearrange("p (n d) -> p n d", d=D),
                in_ap=embed_d[ch * CHUNK:hi, :],
                idxs_ap=gidx_t[:, m["pos"] // 16:(m["pos"] + n) // 16],
                num_idxs=n, num_idxs_reg=n, elem_size=D)
            stages[ci] = st

        def emit_scatter(ci):
            m = callmeta[ci]
            st = stages[ci]
            for sub in m["subs"]:
                n = sub["n"]
                rel = sub["pos"] - m["pos"]
                hs = sub["half"] * HSL
                if sub["kind"] == 0:
                    own = cacc[sub["set"]][0][:, hs:hs + HSL]
                    peer = cacc[sub["set"]][1][:, hs:hs + HSL]
                else:
                    own = qacc[0][:, hs:hs + HSL]
                    peer = qacc[1][:, hs:hs + HSL]
                nc.gpsimd.dma_scatter_add(
                    out_ap=own.rearrange("p (n d) -> p n d", d=D),
                    in_ap=st[:, rel:rel + n]
                        .rearrange("p (n d) -> p n d", d=D),
                    idxs_ap=sidx_t[:, sub["pos"] // 16:(sub["pos"] + n) // 16],
                    num_idxs=n, num_idxs_reg=n, elem_size=D,
                    sbuf_tokens_per_rank=128, parity_reg=0,
                    out_ap_other=peer.rearrange("p (n d) -> p n d", d=D))
            stages[ci] = None

        LOOKAHEAD = 2
        for ci in range(ncalls):
            emit_gather(ci)
            if ci >= LOOKAHEAD:
                emit_scatter(ci - LOOKAHEAD)
        for ci in range(max(0, ncalls - LOOKAHEAD), ncalls):
            emit_scatter(ci)

        # ---- compute (identical math to v1, c_sum = A+B sets) -------------
        for pj in range(PAIRS):
            par = pj % 2
            hf = pj // 32
            grp = (pj - 32 * hf) // 2 + 17 * hf
            csl = slice(grp * 128, (grp + 1) * 128)
            c_sum2 = spool.tile([128, 128], f32, tag="c_sum2")
            nc.vector.tensor_add(c_sum2[:], cacc[0][par][:, csl],
                                 cacc[1][par][:, csl])
            q_h2 = qacc[par][:, csl]

            c_h2 = spool.tile([128, 128], f32, tag="c_h2")
            nc.vector.tensor_scalar_mul(c_h2[:], c_sum2[:],
                                        cscale_t[:, pj:pj + 1])

            t1 = ppool.tile([128, 128], f32, tag="ps_t1")
            nc.tensor.transpose(t1[:], c_h2[:], ident[:])
            c_hT = spool.tile([128, 128], f32, tag="c_hT")
            nc.vector.tensor_copy(c_hT[:], t1[:])

            t2 = ppool.tile([128, 128], f32, tag="ps_t2")
            nc.tensor.transpose(t2[:], q_h2, ident[:])
            q_hT = spool.tile([128, 128], f32, tag="q_hT")
            nc.scalar.copy(q_hT[:], t2[:])

            sim = ppool.tile([128, 128], f32, tag="ps_sim")
            nc.tensor.matmul(sim[:], lhsT=c_hT[:], rhs=q_hT[:],
                             start=True, stop=False)
            nc.tensor.matmul(sim[:], lhsT=blockind_t[:],
                             rhs=qbias_t[:, pj * 128:(pj + 1) * 128],
                             start=False, stop=True)

            att_e = spool.tile([128, 128], f32, tag="att_e")
            s_col = spool.tile([128, 1], f32, tag="s_col")
            nc.scalar.activation(att_e[:], sim[:], Act.Exp,
                                 scale=SCALE_SIM, accum_out=s_col[:])
            r_col = spool.tile([128, 1], f32, tag="r_col")
            nc.vector.reciprocal(r_col[:], s_col[:])
            att = spool.tile([128, 128], f32, tag="att")
            nc.vector.tensor_scalar_mul(att[:], att_e[:], r_col[:])

            t3 = ppool.tile([128, 128], f32, tag="ps_t3")
            nc.tensor.transpose(t3[:], att[:], ident[:])
            attT = spool.tile([128, 128], f32, tag="attT")
            nc.scalar.copy(attT[:], t3[:])

            wq_ps = ppool.tile([128, 128], f32, tag="ps_wq")
            nc.tensor.matmul(wq_ps[:], lhsT=q_h2, rhs=attT[:],
                             start=True, stop=True)
            wqT = spool.tile([128, 128], f32, tag="wqT")
            nc.vector.tensor_copy(wqT[:], wq_ps[:])

            sm_ps = ppool.tile([128, 16], f32, tag="ps_small")
            qs_ps = sm_ps[:, 0:2]
            nc.tensor.matmul(qs_ps, lhsT=q_h2,
                             rhs=qv_t[:, pj * 2:(pj + 1) * 2],
                             start=True, stop=True)
            qs_sb = spool.tile([128, 2], f32, tag="qs_sb")
            nc.vector.tensor_copy(qs_sb[:], qs_ps)

            bias_ps = sm_ps[:, 4:6]
            nc.tensor.matmul(bias_ps, lhsT=whk[:, 0:128], rhs=qs_sb[:],
                             start=True, stop=True)
            bias_sb = spool.tile([128, 2], f32, tag="bias_sb")
            nc.scalar.activation(bias_sb[:], bias_ps, Act.Identity,
                                 bias=b_h_t[:, 0:1])

            ch3 = spool.tile([128, 128], f32, tag="ch3")
            nc.vector.tensor_mul(ch3[:], c_hT[:], wqT[:])
            dif = spool.tile([128, 128], f32, tag="dif")
            nc.vector.tensor_sub(dif[:], c_hT[:], wqT[:])
            ch4 = spool.tile([128, 128], f32, tag="ch4")
            nc.scalar.activation(ch4[:], dif[:], Act.Abs)

            h_ps = ppool.tile([128, 128], f32, tag="ps_h")
            for k, rhs in ((1, c_hT), (2, wqT), (3, ch3), (4, ch4)):
                nc.tensor.matmul(h_ps[:], lhsT=whk[:, k * 128:(k + 1) * 128],
                                 rhs=rhs[:], start=(k == 1), stop=(k == 4))
            hT = spool.tile([128, 128], f32, tag="hT")
            for r in range(2):
                nc.scalar.activation(hT[:, r * 64:(r + 1) * 64],
                                     h_ps[:, r * 64:(r + 1) * 64], Act.Tanh,
                                     bias=bias_sb[:, r:r + 1])

            o_ps = sm_ps[:, 8:9]
            nc.tensor.matmul(o_ps, lhsT=hT[:], rhs=w_o_t[:],
                             start=True, stop=True)
            nc.scalar.activation(out_sb[:, pj:pj + 1], o_ps, Act.Identity,
                                 bias=b_o_t[:, 0:1])

        ot_ps = ppool.tile([PAIRS, 128], f32, tag="ps_ot")
        nc.tensor.transpose(ot_ps[:], out_sb[:], ident[:])
        out_f = const.tile([PAIRS, 128], f32)
        nc.vector.tensor_copy(out_f[:], ot_ps[:])
        nc.sync.dma_start(out_d[:], out_f[:])

    nc.compile()
    return nc


def make_in_maps(q_ids, c_ids, num_qs, num_cols, embed, W_h, b_h, W_o, b_o):
    embed = np.ascontiguousarray(np.asarray(embed, np.float32))
    W_h = np.ascontiguousarray(np.asarray(W_h, np.float32))
    W_o = np.ascontiguousarray(np.asarray(W_o, np.float32).reshape(D, 1))
    b_h = np.ascontiguousarray(np.asarray(b_h, np.float32).reshape(D, 1))
    b_o_bc = np.full((D, 1), np.float32(np.asarray(b_o).reshape(-1)[0]))
    shared = dict(embed=embed, w_h=W_h, w_o=W_o, b_h=b_h, b_o_bc=b_o_bc,
                  blockind=_BLOCKIND)
    percore, callmeta, total = prep_all(q_ids, c_ids, num_qs)
    in_maps = [dict(shared, **percore[i]) for i in range(NCORES)]
    return in_maps, callmeta, total


_PROGRAM = None
_PROGKEY = None


def kernel(q_ids, c_ids, num_qs, num_cols, embed, W_h, b_h, W_o, b_o):
    global _PROGRAM, _PROGKEY
    in_maps, callmeta, total = make_in_maps(q_ids, c_ids, num_qs, num_cols,
                                            embed, W_h, b_h, W_o, b_o)
    key = (len(callmeta), total)
    if _PROGRAM is None or _PROGKEY != key:
        _PROGRAM = _build_program(callmeta, total)
        _PROGKEY = key
    from concourse import bass_utils
    res = bass_utils.run_bass_kernel_spmd(
        _PROGRAM, in_maps, core_ids=list(range(NCORES)), trace=False)
    outs = np.empty((B, C, 1), np.float32)
    for i in range(NCORES):
        outs[i * BL:(i + 1) * BL, :, 0] = res.results[i]["out"].reshape(BL, C)
    return outs
